# revision 1
# baseline (speedup 1.0000x reference)
"""Multi-head attention (B=8, N=1024, D=1024, H=16) on 8 TRN2 NeuronCores.

Sharding: data-parallel over batch - core i computes batch item i end-to-end.

v2 vs v1: x loads first (was: weights first, 19us PE stall); all matmuls
bf16 (f32r ran at 258ns/instr vs bf16's 217 due to LDWEIGHTS bubbles);
S(pair0,c0) is interleaved into the V-projection so the scalar engine's exp
stream starts early; output projection accumulates ct=0..6 into borrowed
PSUM banks during pair 7 so only the tail waits on the last normalization.
(fp8e4 DoubleRow qk-proj was tried: 0.76x PE time but 2.2e-2 rel err - over
the 2e-2 gate - so bf16 everywhere.)

  A)  x -> scalar cast bf16 -> PE transpose (bf16, PSUM bf16) -> xT
  B)  qk-proj: bf16 matmuls w_qk^T xT -> qT,kT stored bf16
  C)  V-proj (bf16, j-outer, 2-bank passes) with S(pair0,c0)+exp interleaved
  D)  per pair: S^T = kT^T qT (bf16 row-tiled pairs), exp (scalar),
      O'^T = V'^T expS; ones col in V' (via whole-tile memset 1.0) gives
      softmax sums on row 64
  E)  normalize: sums -> DRAM -> batched reciprocals -> partition-broadcast
      DMAs -> one [128,1024] mul per pair
  F)  out-proj bf16, partial-ct accumulation, halves DMA'd as they finish
"""

import os
import sys
import types

sys.path.insert(0, "/opt/trn_rl_repo")

if "antenv.axon_hooks" not in sys.modules:
    _hooks = types.ModuleType("antenv.axon_hooks")
    _hook_store = [None]
    _hooks.set_axon_ntff_profile_hook = lambda h: _hook_store.__setitem__(0, h)
    _hooks.get_axon_ntff_profile_hook = lambda: _hook_store[0]
    sys.modules["antenv.axon_hooks"] = _hooks
    try:
        from trn_agent_boot.trn_boot import _ntff_profile_via_ctypes

        _hooks.set_axon_ntff_profile_hook(
            _ntff_profile_via_ctypes("/opt/axon/libaxon_pjrt.so")
        )
    except Exception:
        pass

import numpy as np
import concourse.bass as bass
import concourse.bacc as bacc
import concourse.mybir as mybir
import concourse.tile as tile
from concourse import masks
from concourse.bass_utils import run_bass_kernel_spmd

F32 = mybir.dt.float32
BF16 = mybir.dt.bfloat16
FP8 = mybir.dt.float8e4
EXP = mybir.ActivationFunctionType.Exp
DR = mybir.MatmulPerfMode.DoubleRow

B = 8
N = 1024
D = 1024
H = 16
HD = 64
SCALE = HD**-0.5
NT = N // 128
DT = D // 128
NC2 = N // 512

LAST_EXEC_NS = [None]


def build():
    nc = bacc.Bacc(None, target_bir_lowering=False)
    x = nc.declare_dram_parameter("x", [N, D], F32, isOutput=False)
    w_qkv = nc.declare_dram_parameter("w_qkv", [D, 3 * D], F32, isOutput=False)
    w_proj = nc.declare_dram_parameter("w_proj", [D, D], F32, isOutput=False)
    b_proj = nc.declare_dram_parameter("b_proj", [D], F32, isOutput=False)
    out = nc.declare_dram_parameter("out", [N, D], F32, isOutput=True)

    from contextlib import ExitStack

    with tile.TileContext(nc) as tc:
        with ExitStack() as stack:
            ep = stack.enter_context
            cpool = ep(tc.tile_pool(name="const", bufs=1))
            xTpool = ep(tc.tile_pool(name="xT", bufs=DT))
            Vpool = ep(tc.tile_pool(name="V", bufs=NT))
            qkTpool = ep(tc.tile_pool(name="qkT", bufs=4))
            Opool = ep(tc.tile_pool(name="On", bufs=DT))
            espool = ep(tc.tile_pool(name="es", bufs=10))
            wq8pool = ep(tc.tile_pool(name="wq8", bufs=4))
            wqspool = ep(tc.tile_pool(name="wqs", bufs=2))
            wvpool = ep(tc.tile_pool(name="wv", bufs=DT))
            wppool = ep(tc.tile_pool(name="wp", bufs=DT))
            srowpool = ep(tc.tile_pool(name="srow", bufs=4))
            sumspool = ep(tc.tile_pool(name="sums", bufs=1))
            rbpool = ep(tc.tile_pool(name="recb", bufs=2))
            osbpool = ep(tc.tile_pool(name="osb", bufs=3))
            drpool = ep(tc.tile_pool(name="drs", bufs=1, space="DRAM"))
            pqpool = ep(tc.tile_pool(name="pq", bufs=2, space="PSUM"))
            ident_b = cpool.tile([128, 128], BF16, tag="identb")
            identf = cpool.tile([128, 128], F32, tag="identf")
            masks.make_identity(nc, identf[:])
            nc.vector.tensor_copy(ident_b[:], identf[:])
            b_bc = cpool.tile([128, D], F32, tag="b_bc")
            nc.sync.dma_start(
                b_bc[:].rearrange("p (a f) -> p a f", a=1),
                b_proj[:].rearrange("(a n) -> a n", a=1).partition_broadcast(128),
            )

            xT = [xTpool.tile([128, N], BF16, tag="xT", name=f"xT{j}") for j in range(DT)]
            V = [
                Vpool.tile([128, H * (HD + 1)], BF16, tag="V", name=f"V{i}")
                for i in range(NT)
            ]
            Onorm = [
                Opool.tile([128, N], BF16, tag="On", name=f"On{i}") for i in range(DT)
            ]
            # ones column of V' comes free: memset whole tile to 1.0; the
            # V-proj casts overwrite the 64 value cols per head, col 64 stays 1
            for i in range(NT):
                nc.gpsimd.memset(V[i][:], 1.0)

            # ---------- normalization helpers ----------
            _st_rows = (8, 8, 8, 4, 2, 2)
            _st_off = (0, 8, 16, 24, 28, 30)
            sums_st = [
                sumspool.tile([_st_rows[k], 512], F32, tag=f"sums{k}", name=f"sums{k}")
                for k in range(6)
            ]
            rec_st = [
                sumspool.tile([_st_rows[k], 512], F32, tag=f"rec{k}", name=f"rec{k}")
                for k in range(6)
            ]
            sums_dram = drpool.tile([32, 512], F32, tag="sumsd")
            rec_dram = drpool.tile([32, 512], F32, tag="recd")
            sums_by_c = sums_dram[:].rearrange("(h c) f -> c h f", c=2)
            rec_by_c = rec_dram[:].rearrange("(h c) f -> c h f", c=2)

            def recip_stage(stage):
                r0, nr = _st_off[stage], _st_rows[stage]
                nc.sync.dma_start(sums_st[stage][:], sums_dram[r0 : r0 + nr, :])
                nc.vector.reciprocal(rec_st[stage][:], sums_st[stage][:])
                nc.sync.dma_start(rec_dram[r0 : r0 + nr, :], rec_st[stage][:])

            def recip_pair7_chunk(cidx):
                st = 4 + cidx
                nc.sync.dma_start(sums_st[st][:], sums_by_c[cidx, 14:16, :])
                nc.vector.reciprocal(rec_st[st][:], sums_st[st][:])
                nc.sync.dma_start(rec_by_c[cidx, 14:16, :], rec_st[st][:])

            def mul_pair(pr, chunks=(0, 1)):
                recb = rbpool.tile([128, N], F32, tag="recb")
                for parity in range(2):
                    h = 2 * pr + parity
                    p0 = 64 * parity
                    for c in chunks:
                        hc = h * 2 + c
                        nc.sync.dma_start(
                            recb[p0 : p0 + 64, c * 512 : (c + 1) * 512].rearrange(
                                "p (a f) -> p a f", a=1
                            ),
                            rec_dram[hc : hc + 1, :].partition_broadcast(64),
                        )
                if chunks == (0, 1):
                    nc.vector.tensor_mul(Onorm[pr][:], Onorm[pr][:], recb[:])
                else:
                    c = chunks[0]
                    cs = slice(c * 512, (c + 1) * 512)
                    nc.vector.tensor_mul(
                        Onorm[pr][:, cs], Onorm[pr][:, cs], recb[:, cs]
                    )

            # ---------- qk projection emitter (bf16; fp8 DR measured
            # 2.2e-2 rel err in sim - over the 2e-2 gate - so bf16 it is) ----
            def make_qk_emitter(pr):
                wqs = []
                # all wq casts on DVE: Pool casts (3.5us/tile) stalled
                # the PE mid-C waiting for qk weights
                cast_eng = nc.vector
                for ei, et in enumerate((pr, DT + pr)):
                    stg = wqspool.tile([128, 1024], F32, tag="wqs", name=f"wqs{et}")
                    # DRAM [1024 d, 128 e] -> SBUF [128 p, (j 8, e 128)]
                    nc.sync.dma_start(
                        stg[:].rearrange("p (j e) -> p j e", j=8),
                        w_qkv[:, et * 128 : (et + 1) * 128].rearrange(
                            "(j p) e -> p j e", p=128
                        ),
                    )
                    wq = wq8pool.tile([128, 1024], BF16, tag="wq8", name=f"wq{pr}_{ei}")
                    cast_eng.tensor_copy(wq[:], stg[:])
                    wqs.append(wq)
                qts = [
                    qkTpool.tile([128, N], BF16, tag="qkT", name=f"qt{pr}"),
                    qkTpool.tile([128, N], BF16, tag="qkT", name=f"kt{pr}"),
                ]
                state = {"idx": 0, "pq": None}

                def emit(n):
                    # 32 matmuls: 2 etiles x 2 chunks x 8 j
                    for _ in range(n):
                        idx = state["idx"]
                        if idx >= 32:
                            return
                        g, j = idx // 8, idx % 8
                        ei, c = g // 2, g % 2
                        if j == 0:
                            state["pq"] = pqpool.tile(
                                [128, 512], F32, tag="pq", name="pq"
                            )
                        wqv = wqs[ei][:].rearrange("p (j e) -> p j e", j=8)
                        nc.tensor.matmul(
                            state["pq"][:],
                            wqv[:, j, :],
                            xT[j][:, c * 512 : (c + 1) * 512],
                            start=(j == 0),
                            stop=(j == 7),
                        )
                        if j == 7:
                            nc.vector.tensor_copy(
                                qts[ei][:, c * 512 : (c + 1) * 512], state["pq"][:]
                            )
                        state["idx"] = idx + 1

                return emit, qts

            # ============ phase A: load x, cast, transpose ============
            with (
                tc.tile_pool(name="xin", bufs=2) as xspool,
                tc.tile_pool(name="xbf", bufs=2) as xbfpool,
                tc.tile_pool(name="wvs", bufs=2) as wvspool,
                tc.tile_pool(name="tp", bufs=3, space="PSUM") as tppool,
            ):
                for i in range(NT):
                    xs = xspool.tile([128, D], F32, tag="xs")
                    nc.sync.dma_start(xs[:], x[i * 128 : (i + 1) * 128, :])
                    xb = xbfpool.tile([128, D], BF16, tag="xb")
                    nc.scalar.copy(xb[:], xs[:])
                    for j in range(DT):
                        tp = tppool.tile([128, 128], BF16, tag="tp")
                        nc.tensor.transpose(
                            tp[:], xb[:, j * 128 : (j + 1) * 128], ident_b[:]
                        )
                        # GPSIMD cannot read PSUM; transpose copies stay on DVE
                        nc.vector.tensor_copy(xT[j][:, i * 128 : (i + 1) * 128], tp[:])

                # w_v staging (DMA queues behind x) + bf16 casts on DVE
                wv = []
                for j in range(DT):
                    stg = wvspool.tile([128, D], F32, tag="wvs")
                    nc.sync.dma_start(
                        stg[:], w_qkv[j * 128 : (j + 1) * 128, 2 * D : 3 * D]
                    )
                    t = wvpool.tile([128, D], BF16, tag="wv", name=f"wv{j}")
                    nc.vector.tensor_copy(t[:], stg[:])
                    wv.append(t)

            # ============ mega phase: qk0 + V-proj + S(pair0,c0) ============
            wp = []
            pd_borrow = {}
            es_p0 = {}

            with tc.tile_pool(name="s", bufs=2, space="PSUM") as spool:
                with tc.tile_pool(name="vp", bufs=2, space="PSUM") as vppool:
                    emit0, qts0 = make_qk_emitter(0)
                    emit0(32)
                    qt0, kt0 = qts0

                    s_state = {"idx": 0}

                    def emit_s_p0(nsteps):
                        # S + exp for pair 0 chunk 0 only (8 j-steps)
                        for _ in range(nsteps):
                            idx = s_state["idx"]
                            if idx >= NT:
                                return
                            j = idx
                            st = spool.tile([128, N], F32, tag="s", name="st")
                            es = espool.tile([128, N], BF16, tag="es", name=f"e0{idx}")
                            for parity in range(2):
                                p0 = 64 * parity
                                nc.tensor.matmul(
                                    st[:, 512 * parity : 512 * parity + 512],
                                    kt0[p0 : p0 + 64, j * 128 : (j + 1) * 128],
                                    qt0[p0 : p0 + 64, 0:512],
                                    start=True,
                                    stop=True,
                                )
                            nc.scalar.activation(es[:], st[:], EXP, scale=SCALE)
                            es_p0[j] = es
                            s_state["idx"] = idx + 1

                    # V-proj: j-outer passes over i-pairs; 2 PSUM accumulators
                    for c in range(NC2):
                        for g in range(4):
                            pv = {}
                            for i in (2 * g, 2 * g + 1):
                                pv[i] = vppool.tile(
                                    [128, 512], F32, tag="vp", name=f"vp{i}"
                                )
                            for j in range(DT):
                                for i in (2 * g, 2 * g + 1):
                                    nc.tensor.matmul(
                                        pv[i][:],
                                        xT[j][:, i * 128 : (i + 1) * 128],
                                        wv[j][:, c * 512 : (c + 1) * 512],
                                        start=(j == 0),
                                        stop=(j == DT - 1),
                                    )
                            for i in (2 * g, 2 * g + 1):
                                dst = V[i][:].rearrange("p (h e) -> p h e", e=HD + 1)
                                nc.vector.tensor_copy(
                                    dst[:, 8 * c : 8 * c + 8, 0:HD],
                                    pv[i][:].rearrange("p (h e) -> p h e", e=HD),
                                )
                            emit_s_p0(1)
                    emit_s_p0(NT)

                # ============ phase C: attention pairs ============
                with tc.tile_pool(name="o", bufs=2, space="PSUM") as opool:

                    def emit_dhead(ct, ec):
                        # borrowed-bank out-proj head start: i=0, ct 0..6
                        if (0, ec) not in pd_borrow:
                            pd_borrow[(0, ec)] = pqpool.tile(
                                [128, 512], F32, tag="pq", name=f"dh{ec}"
                            )
                        nc.tensor.matmul(
                            pd_borrow[(0, ec)][:],
                            Onorm[ct][:, 0:128],
                            wp[ct][:, ec * 512 : (ec + 1) * 512],
                            start=(ct == 0),
                            stop=False,
                            skip_group_check=True,
                        )

                    cur_qts = qts0
                    pending_recips = []
                    for pair in range(H // 2):
                        if pair in (2, 3, 4, 5):
                            # w_proj load+cast, spread over 4 pairs to avoid
                            # hogging the DMA queue; casts on DVE (Pool slow)
                            for ct in range(2 * (pair - 2), 2 * (pair - 2) + 2):
                                stg = wqspool.tile(
                                    [128, D], F32, tag="wqs", name=f"wps{ct}"
                                )
                                nc.sync.dma_start(
                                    stg[:], w_proj[ct * 128 : (ct + 1) * 128, :]
                                )
                                t = wppool.tile([128, D], BF16, tag="wp", name=f"wp{ct}")
                                nc.vector.tensor_copy(t[:], stg[:])
                                wp.append(t)
                        if pair + 1 < H // 2:
                            emit_next, next_qts = make_qk_emitter(pair + 1)
                        else:
                            emit_next, next_qts = (lambda n: None), None
                            # rec_dram readers rely on queue FIFO order: flush
                            # deferred recips before the muls that read them
                            while pending_recips:
                                recip_stage(pending_recips.pop(0))
                            mul_pair(5)
                            mul_pair(6)
                        qt_pair, kt_pair = cur_qts

                        dhead = []
                        if pair == H // 2 - 1:
                            dhead = [(ct, ec) for ec in range(2) for ct in range(7)]

                        for cpass in range(NC2):
                            cs = slice(cpass * 512, (cpass + 1) * 512)
                            po = [
                                opool.tile([128, 512], F32, tag="o", name=f"po{p}")
                                for p in range(2)
                            ]
                            for j in range(NT):
                                if pair == 0 and cpass == 0:
                                    es = es_p0[j]
                                else:
                                    st = spool.tile([128, N], F32, tag="s", name="st")
                                    es = espool.tile([128, N], BF16, tag="es")
                                    for parity in range(2):
                                        p0 = 64 * parity
                                        nc.tensor.matmul(
                                            st[:, 512 * parity : 512 * parity + 512],
                                            kt_pair[
                                                p0 : p0 + 64, j * 128 : (j + 1) * 128
                                            ],
                                            qt_pair[p0 : p0 + 64, cs],
                                            start=True,
                                            stop=True,
                                        )
                                    nc.scalar.activation(es[:], st[:], EXP, scale=SCALE)
                                for parity in range(2):
                                    h = 2 * pair + parity
                                    nc.tensor.matmul(
                                        po[parity][0 : HD + 1, :],
                                        V[j][:, h * (HD + 1) : (h + 1) * (HD + 1)],
                                        es[:, 512 * parity : 512 * parity + 512],
                                        start=(j == 0),
                                        stop=(j == NT - 1),
                                    )
                                emit_next(2)
                                if dhead and j % 2 == 1:
                                    emit_dhead(*dhead.pop(0))
                            for parity in range(2):
                                h = 2 * pair + parity
                                p0 = 64 * parity
                                nc.vector.tensor_copy(
                                    Onorm[pair][p0 : p0 + 64, cs], po[parity][0:HD, :]
                                )
                                hc = h * 2 + cpass
                                srow = srowpool.tile([1, 512], F32, tag="srow")
                                nc.vector.tensor_copy(srow[:], po[parity][HD : HD + 1, :])
                                nc.sync.dma_start(sums_dram[hc : hc + 1, :], srow[:])
                            if cpass == 0 and pending_recips:
                                while pending_recips:
                                    recip_stage(pending_recips.pop(0))
                            if pair == H // 2 - 1 and cpass == 0:
                                recip_pair7_chunk(0)
                                mul_pair(7, chunks=(0,))
                        while dhead:
                            emit_dhead(*dhead.pop(0))
                        cur_qts = next_qts
                        if pair % 2 == 1 and pair < 7:
                            pending_recips.append(pair // 2)
                        if pair == 6:
                            pending_recips.append(3)
                        if pair >= 2 and pair < 7:
                            mul_pair(pair - 2)

                    # finish borrowed i=0 accumulators: ct7 needs only the
                    # chunk-0 half of Onorm[7] (cols 0:128), normalized above
                    for ec in range(2):
                        nc.tensor.matmul(
                            pd_borrow[(0, ec)][:],
                            Onorm[7][:, 0:128],
                            wp[7][:, ec * 512 : (ec + 1) * 512],
                            start=False,
                            stop=True,
                            skip_group_check=True,
                        )
                        ob = osbpool.tile([128, 512], F32, tag="osbh", name=f"ob{ec}")
                        nc.vector.tensor_add(
                            ob[:], pd_borrow[(0, ec)][:], b_bc[:, ec * 512 : (ec + 1) * 512]
                        )
                        nc.sync.dma_start(
                            out[0:128, ec * 512 : (ec + 1) * 512], ob[:]
                        )

            recip_pair7_chunk(1)
            mul_pair(7, chunks=(1,))

            # ============ phase D: output projection + bias ============
            with tc.tile_pool(name="dp", bufs=6, space="PSUM") as dppool:
                for group in ([1, 2, 3], [4, 5, 6], [7]):
                    pd = {}
                    for i in group:
                        for ec in range(2):
                            pd[(i, ec)] = dppool.tile(
                                [128, 512], F32, tag="dp", name=f"dp{i}_{ec}"
                            )
                    for ct in range(DT):
                        for i in group:
                            for ec in range(2):
                                nc.tensor.matmul(
                                    pd[(i, ec)][:],
                                    Onorm[ct][:, i * 128 : (i + 1) * 128],
                                    wp[ct][:, ec * 512 : (ec + 1) * 512],
                                    start=(ct == 0),
                                    stop=(ct == DT - 1),
                                )
                    for i in group:
                        ob = osbpool.tile([128, D], F32, tag="osb", name=f"ob{i}")
                        for ec in range(2):
                            nc.vector.tensor_add(
                                ob[:, ec * 512 : (ec + 1) * 512],
                                pd[(i, ec)][:],
                                b_bc[:, ec * 512 : (ec + 1) * 512],
                            )
                        qeng = nc.sync if i % 2 == 0 else nc.scalar
                        qeng.dma_start(out[i * 128 : (i + 1) * 128, :], ob[:])

    nc.compile()
    return nc


_NC = [None]


def _get_nc():
    if _NC[0] is None:
        _NC[0] = build()
    return _NC[0]


def kernel(x, w_qkv, w_proj, b_proj):
    x = np.asarray(x, dtype=np.float32)
    w_qkv = np.asarray(w_qkv, dtype=np.float32)
    w_proj = np.asarray(w_proj, dtype=np.float32)
    b_proj = np.asarray(b_proj, dtype=np.float32)
    assert x.shape == (B, N, D)

    nc = _get_nc()
    in_maps = [
        {"x": x[i], "w_qkv": w_qkv, "w_proj": w_proj, "b_proj": b_proj}
        for i in range(B)
    ]
    trace = os.environ.get("KERNEL_TRACE") == "1"
    res = run_bass_kernel_spmd(nc, in_maps, core_ids=list(range(B)), trace=trace)
    LAST_EXEC_NS[0] = res.exec_time_ns
    return np.stack([res.results[i]["out"] for i in range(B)], axis=0)

